# revision 29
# baseline (speedup 1.0000x reference)
"""AutoCorrelation (Autoformer-style) Bass kernel for one TRN2 chip (8 NeuronCores).

Math: the reference computes, per (b, h):
    corr = irfft(rfft(q, axis=-1) * conj(rfft(k, axis=-1)), n=L)   # [L, L]
    weights = softmax(corr - mean_h(corr), axis=-1)
    Vt = v @ weights                                                # [d, L]
The rfft runs over the d=64 channel axis and the irfft zero-pads 33 bins to
L=2048, so corr[s, :] is a rank-<=66 function of t; the DC term is constant
over t and cancels in softmax.  Collapsing the spectral products
(re*re + im*im -> cos row, im*re - re*im -> sin row) leaves 64 coefficient
rows: the logits are an exact K=64 matmul against a fixed cos/sin basis and
no [L, L] tensor ever exists in DRAM.

Split of labour: the coefficient pipeline (a [*, 64] x [64, 128] spectrum
transform of q/k, an elementwise product, a fold, and the head-mean
subtraction -- ~2 GFLOP total) runs on the host in fp32 as part of input
sharding; it feeds a single per-core NEFF that does softmax + delay
aggregation for one head (both batches).  The head-mean is the only
cross-head coupling and it dissolves into the host prep, so no collective
and no second NEFF launch is needed.

Device kernel, per (b, chunk of 128 s-rows):
  - logits [128, 2048] = cd-chunk^T @ basis as 4 K=64 matmuls (PE)
  - exp: t-half 0 on ScalarE (table exp, fused row-sum), t-half 1 on
    VectorE (custom DVE op EXP8_ANT: exp(x) ~= (c0 + x(c1 + x c2))^8,
    valid since logits are bounded by ~1.5; fused row-sum)
  - 1/rowsum folds into the tiny v-tile (gpsimd add, vector reciprocal,
    scalar copy-with-scale), not the [128, 2048] weight tile
  - delay aggregation accumulates in PSUM as column-packed matmul pairs
    (out partitions 0:64 = t 0:1024, 64:128 = t 1024:2048) which co-run
    on separate PE column groups.
Aggregation matmuls for chunk sc-2 are emitted BEFORE the logits of chunk
sc so a stalled logits matmul never blocks ready aggregation work behind
it in the in-order PE queue.
"""
import sys
from operator import add as _op_add

sys.path.insert(0, "/opt/trn_rl_repo")

import numpy as np
import ml_dtypes

from concourse import bass, bacc, mybir, tile
from concourse import dve_ops
from concourse.dve_spec import Spec, Src0, C0, C1, C2, Zero, sq, lower
from concourse.dve_uop import DveOpSpec
from concourse.bass_utils import run_bass_kernel_spmd

B, L, E, H, D = 2, 2048, 512, 8, 64
NF = 32          # frequencies 1..32 of the 64-point rfft (DC dropped)
NCC = 2 * NF     # 64 compressed coefficient rows (cos, sin)
NCORES = 8
SC = L // 128    # 16 s-chunks of 128 rows
BF16 = mybir.dt.bfloat16
F32 = mybir.dt.float32

# minimax quadratic p(z) for e^z on z = x/8, |x| <= 1.68; exp(x) ~= p(x)^8
EXP_C = (0.99970171, 0.12580122, 0.00795605)

TRACE = False
LAST_RESULT = None

_COMPILED_B = None
_EXP_OP = None
_CONSTS = None


def _register_exp_op():
    global _EXP_OP
    if _EXP_OP is not None:
        return _EXP_OP
    for o in dve_ops.OPS:
        if o.name == "EXP8_ANT":
            _EXP_OP = o
            return o

    body = sq(sq(sq(C0 + Src0 * (C1 + Src0 * C2))))

    def _ref(in0, in1, c0, c1, c2):
        x = in0.astype(np.float32)
        b = (((c0 + x * (c1 + x * c2)) ** 8)).astype(np.float32)
        return b, b.reshape(b.shape[0], -1).sum(axis=-1, keepdims=True)

    spec = Spec(body=body, accum=_op_add, accum_init=Zero, reference=_ref)
    opcode = dve_ops._CUSTOM_DVE_ROW_BASE + len(dve_ops.OPS)
    dve_ops._SUB_OPCODE_FOR_NAME["EXP8_ANT"] = opcode
    shas = {}
    for ver in ("v3", "v4"):
        shas[ver] = DveOpSpec(
            name="EXP8_ANT", opcode=opcode, uops=lower(spec, ver=ver), rd1_en=False
        ).sha(ver)
    op = dve_ops.DveOp("EXP8_ANT", spec, subdim=False, uops_sha=shas)
    dve_ops.OPS.append(op)
    dve_ops.CUSTOM_DVE_SPECS[op.name] = spec
    _EXP_OP = op
    return op


def _constants():
    """fx/fy spectrum transforms (fp32, host) and cos/sin basis (bf16, device)."""
    global _CONSTS
    if _CONSTS is not None:
        return _CONSTS
    c = np.arange(D)
    f = np.arange(1, NF + 1)
    ang = 2 * np.pi * np.outer(c, f) / D
    fcos = np.cos(ang).astype(np.float32)    # Re X_f   = sum_c q_c cos
    fsin = (-np.sin(ang)).astype(np.float32)  # Im X_f  = -sum_c q_c sin
    w = np.float32(2.0 / L)                  # irfft weight for interior bins
    fx = np.concatenate([fcos * w, fsin * w, fsin * w, fcos * w], axis=1)  # [64,128]
    fy = np.concatenate([fcos, fsin, fcos, fsin], axis=1)                  # [64,128]
    t = np.arange(L)
    angt = 2 * np.pi * np.outer(f, t) / L
    basis = np.concatenate([np.cos(angt), -np.sin(angt)], axis=0)          # [64, 2048]
    _CONSTS = (fx, fy, basis.astype(ml_dtypes.bfloat16))
    return _CONSTS


def _host_coeffs(queries, keys):
    """Mean-subtracted compressed coefficients cd [B, L, H, 64], fp32.

    logits[b,h,s,t] = sum_k cd[b,s,h,k] * basis[k,t].
    """
    fx, fy, _ = _constants()
    q4 = queries.reshape(B * L * H, D)
    k4 = keys.reshape(B * L * H, D)
    X = q4 @ fx          # [BLH, 128]
    Y = k4 @ fy
    P = X * Y
    # fold: cos rows = P[0:32] + P[32:64] (re*re + im*im),
    #       -sin rows = P[64:96] - P[96:128] (im*re - re*im)
    Ccs = np.empty((B * L * H, NCC), np.float32)
    np.add(P[:, 0:NF], P[:, NF:2 * NF], out=Ccs[:, 0:NF])
    np.subtract(P[:, 2 * NF:3 * NF], P[:, 3 * NF:4 * NF], out=Ccs[:, NF:NCC])
    Ccs = Ccs.reshape(B, L, H, NCC)
    Ccs -= Ccs.mean(axis=2, keepdims=True)
    return Ccs


def _build_b():
    """Softmax + delay aggregation from host-computed coefficients."""
    exp_op = _register_exp_op()
    nc = bacc.Bacc("TRN2", target_bir_lowering=False, debug=False, num_devices=NCORES)
    cd_d = nc.dram_tensor("cd2", [B, 2 * NCC, L], BF16, kind="ExternalInput")
    # v pre-transposed on host to [B, 128, SC*D]: partition-contiguous rows
    # give 2 KB DMA descriptors instead of 128 B ones
    v_d = nc.dram_tensor("vp", [B, 128, SC * D], BF16, kind="ExternalInput")
    basis_d = nc.dram_tensor("basis2", [2 * NCC, L], BF16, kind="ExternalInput")
    out_d = nc.dram_tensor("out", [B, D, L], BF16, kind="ExternalOutput")

    with tile.TileContext(nc) as tc:
        with (
            tc.tile_pool(name="consts", bufs=1) as consts,
            tc.tile_pool(name="vv", bufs=2) as v_pool,
            tc.tile_pool(name="cd", bufs=4) as cd_pool,
            tc.tile_pool(name="wts", bufs=10) as w_pool,
            tc.tile_pool(name="small", bufs=12) as s_pool,
            tc.tile_pool(name="outp", bufs=2) as out_pool,
            tc.tile_pool(name="ps_log", bufs=3, space="PSUM") as ps_log,
            tc.tile_pool(name="ps_vt", bufs=1, space="PSUM") as ps_vt,
        ):
            # DMA issue order = need order.  Sync ring: cd b0 (first logits
            # stationary), basis halves (first logits moving), cd b1.
            # GpSimd ring in parallel: v0, v1.  Whole-tensor transfers with
            # 4 KB per-partition rows keep descriptor count low.
            cdf0 = cd_pool.tile([2 * NCC, L], BF16, tag="cdf0", name="cdf0")
            basis_sb = consts.tile([2 * NCC, L], BF16)
            # gate chunk 0 on as few bytes as possible: cd-b0 cols 0:1024
            # (first 8 chunks' stationaries) + basis half 0 unblock lg0(0);
            # basis half 1 unblocks lg1(0)
            # quarter-granular gate: the first logits matmul needs only
            # cd cols 0:128 and basis cols 0:512 -- 256 KB streams before
            # it can start; later quarters arrive while chunk 0 executes.
            nc.sync.dma_start(out=cdf0[:, 0:512], in_=cd_d[0][:, 0:512])
            nc.sync.dma_start(out=basis_sb[:, 0:512], in_=basis_d[:, 0:512])
            nc.sync.dma_start(out=basis_sb[:, 512:1024], in_=basis_d[:, 512:1024])
            nc.sync.dma_start(out=basis_sb[:, 1024:1536], in_=basis_d[:, 1024:1536])
            nc.sync.dma_start(out=basis_sb[:, 1536:2048], in_=basis_d[:, 1536:2048])
            nc.sync.dma_start(out=cdf0[:, 512:1024], in_=cd_d[0][:, 512:1024])
            nc.sync.dma_start(out=cdf0[:, 1024:2048], in_=cd_d[0][:, 1024:2048])
            # v and cd-b1 go on the same ring AFTER the head-gating tensors
    # so their descriptors don't compete for the 16 hw queues during
            # the first-chunk gate; v0 is first needed at iteration ~1.
            v_sbs = []
            v_sb0 = v_pool.tile([128, SC * D], BF16, tag="v0", name="v_sb0")
            nc.sync.dma_start(out=v_sb0[:], in_=v_d[0])
            v_sbs.append(v_sb0)
            cdf1 = cd_pool.tile([2 * NCC, L], BF16, tag="cdf1", name="cdf1")
            nc.sync.dma_start(out=cdf1[:], in_=cd_d[1][:])
            cd_fulls = [cdf0, cdf1]
            v_sb1 = v_pool.tile([128, SC * D], BF16, tag="v1", name="v_sb1")
            nc.gpsimd.dma_start(out=v_sb1[:], in_=v_d[1])
            v_sbs.append(v_sb1)

            for b in range(B):
                v_sb = v_sbs[b]
                vt_ps = ps_vt.tile([128, 1024], F32, tag="vt")
                wts_hist = {}
                vts_hist = {}
                sig_hist = {}

                def emit_acc(sc):
                    pwt = wts_hist.pop(sc)
                    pvts = vts_hist.pop(sc)
                    for q in range(2):
                        # column-packed pair: PE col groups 0:64 / 64:128
                        # co-run with independent moving streams
                        nc.tensor.matmul(
                            vt_ps[0:D, q * 512:(q + 1) * 512],
                            pvts[:],
                            pwt[0][:, q * 512:(q + 1) * 512],
                            start=(sc == 0), stop=(sc == SC - 1),
                        )
                        nc.tensor.matmul(
                            vt_ps[D:2 * D, q * 512:(q + 1) * 512],
                            pvts[:],
                            pwt[1][:, q * 512:(q + 1) * 512],
                            start=(sc == 0), stop=(sc == SC - 1),
                        )

                def emit_small(sc, fast=False):
                    # fast=True only at the pipeline drain (all exps done):
                    # the whole chain runs on the then-idle DVE so the final
                    # aggregation matmuls don't stall on gpsimd's ~1.5 us
                    # latency.  Never route through DVE mid-stream -- any
                    # op inserted ahead of an exp in the in-order queue
                    # cascades into PE stalls.
                    sig = sig_hist.pop(sc)
                    sigsum = s_pool.tile([128, 1], F32, tag="sigsum")
                    eng = nc.vector if fast else nc.gpsimd
                    eng.tensor_add(sigsum[:], sig[:, 0:1], sig[:, 1:2])
                    rcp = s_pool.tile([128, 1], F32, tag="rcp")
                    nc.vector.reciprocal_approx_fast(rcp[:], sigsum[:])
                    vts = s_pool.tile([128, D], BF16, tag="vts")
                    eng.tensor_scalar_mul(
                        vts[:], v_sb[:, sc * D:(sc + 1) * D], rcp[:]
                    )
                    vts_hist[sc] = vts

                for sc in range(SC):
                    cdf = cd_fulls[b]
                    off = sc * 128
                    cdt = cdf[0:NCC, off:off + 128]
                    cdb = cdf[NCC:2 * NCC, off:off + 128]
                    lg0 = ps_log.tile([128, 1024], F32, tag="log")
                    lg1 = ps_log.tile([128, 1024], F32, tag="log")
                    for q in range(2):
                        # alternating PE row-tiles (0,0)/(64,0) keep the
                        # weight-load pipeline overlapped
                        nc.tensor.matmul(
                            lg0[:, q * 512:(q + 1) * 512], cdt,
                            basis_sb[0:NCC, q * 512:(q + 1) * 512],
                            start=True, stop=True,
                        )
                        nc.tensor.matmul(
                            lg1[:, q * 512:(q + 1) * 512], cdb,
                            basis_sb[NCC:2 * NCC, 1024 + q * 512: 1024 + (q + 1) * 512],
                            start=True, stop=True,
                        )
                    if sc >= 3:
                        # lag 3: vts(sc-3) finished a full iteration ago, so
                        # aggregation never waits on the gpsimd scale chain
                        emit_acc(sc - 3)
                    sig = s_pool.tile([128, 2], F32, tag="sig")
                    wt0 = w_pool.tile([128, 1024], BF16, tag="wt")
                    nc.scalar.activation(
                        wt0[:], lg0[:], mybir.ActivationFunctionType.Exp,
                        accum_out=sig[:, 0:1],
                    )
                    wt1 = w_pool.tile([128, 1024], BF16, tag="wt")
                    nc.vector._custom_dve(
                        exp_op, out=wt1[:], in0=lg1[:],
                        s0=EXP_C[0], s1=EXP_C[1], imm2=EXP_C[2],
                        accum_out=sig[:, 1:2],
                    )
                    wts_hist[sc] = (wt0, wt1)
                    sig_hist[sc] = sig
                    if sc >= 1:
                        emit_small(sc - 1)

                emit_small(SC - 1, fast=True)
                emit_acc(SC - 3)
                emit_acc(SC - 2)
                emit_acc(SC - 1)

                out_sb = out_pool.tile([128, 1024], BF16, tag="out")
                # partition-split copies: each out DMA gates on only one copy
                nc.scalar.copy(out_sb[0:D, :], vt_ps[0:D, :])
                nc.sync.dma_start(out=out_d[b][:, 0:1024], in_=out_sb[0:D, :])
                nc.vector.tensor_copy(out_sb[D:2 * D, :], vt_ps[D:2 * D, :])
                nc.sync.dma_start(out=out_d[b][:, 1024:2048], in_=out_sb[D:2 * D, :])
    nc.compile()
    return nc


def _get_compiled():
    global _COMPILED_B
    if _COMPILED_B is None:
        _COMPILED_B = _build_b()
    return _COMPILED_B


def kernel(queries, keys, values):
    global LAST_RESULT
    queries = np.asarray(queries, dtype=np.float32)
    keys = np.asarray(keys, dtype=np.float32)
    values = np.asarray(values, dtype=np.float32)

    _, _, basis16 = _constants()
    bf = ml_dtypes.bfloat16

    cd = _host_coeffs(queries, keys)                  # [B, L, H, 64] fp32
    cdT = np.ascontiguousarray(cd.transpose(0, 2, 3, 1)).astype(bf)  # [B,H,64,L]
    basis2 = np.concatenate([basis16, basis16], axis=0)              # [128, L]

    # v transposed to [B, 128, SC*D]: vp[b, p, c*D + d] = v[b, c*128 + p, d]
    vp_full = values.reshape(B, SC, 128, E).transpose(0, 2, 1, 3)  # [B,128,SC,E]

    in_maps = []
    for i in range(NCORES):
        sl = slice(i * D, (i + 1) * D)
        cd_i = cdT[:, i]                              # [B, 64, L]
        vp_i = np.ascontiguousarray(vp_full[:, :, :, sl]).astype(bf)
        in_maps.append({
            "cd2": np.concatenate([cd_i, cd_i], axis=1),  # [B, 128, L]
            "vp": vp_i.reshape(B, 128, SC * D),
            "basis2": basis2,
        })

    kw = {"trace_cores": list(range(NCORES))} if TRACE else {}
    ncb = _get_compiled()
    res = run_bass_kernel_spmd(ncb, in_maps, core_ids=list(range(NCORES)),
                               trace=TRACE, **kw)
    LAST_RESULT = res

    vt_full = np.stack([res.results[i]["out"] for i in range(NCORES)], axis=1)
    # reference: out = transpose(Vt[B,H,d,L], (0,2,1,3)).reshape(B, L, H*d)
    return np.ascontiguousarray(
        vt_full.astype(np.float32).transpose(0, 2, 1, 3).reshape(B, L, E)
    )


# revision 30
# speedup vs baseline: 1.0171x; 1.0171x over previous
"""AutoCorrelation (Autoformer-style) Bass kernel for one TRN2 chip (8 NeuronCores).

Math: the reference computes, per (b, h):
    corr = irfft(rfft(q, axis=-1) * conj(rfft(k, axis=-1)), n=L)   # [L, L]
    weights = softmax(corr - mean_h(corr), axis=-1)
    Vt = v @ weights                                                # [d, L]
The rfft runs over the d=64 channel axis and the irfft zero-pads 33 bins to
L=2048, so corr[s, :] is a rank-<=66 function of t; the DC term is constant
over t and cancels in softmax.  Collapsing the spectral products
(re*re + im*im -> cos row, im*re - re*im -> sin row) leaves 64 coefficient
rows: the logits are an exact K=64 matmul against a fixed cos/sin basis and
no [L, L] tensor ever exists in DRAM.

Split of labour: the coefficient pipeline (a [*, 64] x [64, 128] spectrum
transform of q/k, an elementwise product, a fold, and the head-mean
subtraction -- ~2 GFLOP total) runs on the host in fp32 as part of input
sharding; it feeds a single per-core NEFF that does softmax + delay
aggregation for one head (both batches).  The head-mean is the only
cross-head coupling and it dissolves into the host prep, so no collective
and no second NEFF launch is needed.

Device kernel, per (b, chunk of 128 s-rows):
  - logits [128, 2048] = cd-chunk^T @ basis as 4 K=64 matmuls (PE)
  - exp: t-half 0 on ScalarE (table exp, fused row-sum), t-half 1 on
    VectorE (custom DVE op EXP8_ANT: exp(x) ~= (c0 + x(c1 + x c2))^8,
    valid since logits are bounded by ~1.5; fused row-sum)
  - 1/rowsum folds into the tiny v-tile (gpsimd add, vector reciprocal,
    gpsimd scale), not the [128, 2048] weight tile
  - delay aggregation accumulates in PSUM as column-packed matmul pairs
    (out partitions 0:64 = t 0:1024, 64:128 = t 1024:2048) which co-run
    on separate PE column groups.

Scheduling notes (hard-won; the engine queues are in-order and the
pipeline is hypersensitive -- perturbations cascade into ~15 us
regressions via the PE clock dropping to its mid p-state):
  - aggregation runs 3 chunks behind the logits (lag 3) so it never
    waits on the ~1.5 us gpsimd normalization chain (lag 4 is worse:
    longer drain);
  - only the FINAL chunk's normalization chain may route through the
    then-idle DVE; doing this for any earlier chunk (even ones emitted
    after all exps) regresses badly;
  - wt pool holds 10 ring slots (8 live) so exps never block on the PE
    reading a 3-chunks-old weight tile;
  - DMAs are issued in need-order with per-partition rows >= 2 KB (v is
    pre-transposed on the host for this); the first-chunk gate is cd-b0
    cols 0:1024 + basis half 0 (finer quarter-gating adds more
    descriptor overhead than it saves);
  - a 4-uop DVE exp (p(x/4)^4) is speed-neutral and only costs accuracy;
    the DVE is not the critical path.
"""
import sys
from operator import add as _op_add

sys.path.insert(0, "/opt/trn_rl_repo")

import numpy as np
import ml_dtypes

from concourse import bass, bacc, mybir, tile
from concourse import dve_ops
from concourse.dve_spec import Spec, Src0, C0, C1, C2, Zero, sq, lower
from concourse.dve_uop import DveOpSpec
from concourse.bass_utils import run_bass_kernel_spmd

B, L, E, H, D = 2, 2048, 512, 8, 64
NF = 32          # frequencies 1..32 of the 64-point rfft (DC dropped)
NCC = 2 * NF     # 64 compressed coefficient rows (cos, sin)
NCORES = 8
SC = L // 128    # 16 s-chunks of 128 rows
BF16 = mybir.dt.bfloat16
F32 = mybir.dt.float32

# minimax quadratic p(z) for e^z on z = x/8, |x| <= 1.68; exp(x) ~= p(x)^8
EXP_C = (0.99970171, 0.12580122, 0.00795605)

TRACE = False
LAST_RESULT = None

_COMPILED_B = None
_EXP_OP = None
_CONSTS = None


def _register_exp_op():
    global _EXP_OP
    if _EXP_OP is not None:
        return _EXP_OP
    for o in dve_ops.OPS:
        if o.name == "EXP8_ANT":
            _EXP_OP = o
            return o

    body = sq(sq(sq(C0 + Src0 * (C1 + Src0 * C2))))

    def _ref(in0, in1, c0, c1, c2):
        x = in0.astype(np.float32)
        b = (((c0 + x * (c1 + x * c2)) ** 8)).astype(np.float32)
        return b, b.reshape(b.shape[0], -1).sum(axis=-1, keepdims=True)

    spec = Spec(body=body, accum=_op_add, accum_init=Zero, reference=_ref)
    opcode = dve_ops._CUSTOM_DVE_ROW_BASE + len(dve_ops.OPS)
    dve_ops._SUB_OPCODE_FOR_NAME["EXP8_ANT"] = opcode
    shas = {}
    for ver in ("v3", "v4"):
        shas[ver] = DveOpSpec(
            name="EXP8_ANT", opcode=opcode, uops=lower(spec, ver=ver), rd1_en=False
        ).sha(ver)
    op = dve_ops.DveOp("EXP8_ANT", spec, subdim=False, uops_sha=shas)
    dve_ops.OPS.append(op)
    dve_ops.CUSTOM_DVE_SPECS[op.name] = spec
    _EXP_OP = op
    return op


def _constants():
    """fx/fy spectrum transforms (fp32, host) and cos/sin basis (bf16, device)."""
    global _CONSTS
    if _CONSTS is not None:
        return _CONSTS
    c = np.arange(D)
    f = np.arange(1, NF + 1)
    ang = 2 * np.pi * np.outer(c, f) / D
    fcos = np.cos(ang).astype(np.float32)    # Re X_f   = sum_c q_c cos
    fsin = (-np.sin(ang)).astype(np.float32)  # Im X_f  = -sum_c q_c sin
    w = np.float32(2.0 / L)                  # irfft weight for interior bins
    fx = np.concatenate([fcos * w, fsin * w, fsin * w, fcos * w], axis=1)  # [64,128]
    fy = np.concatenate([fcos, fsin, fcos, fsin], axis=1)                  # [64,128]
    t = np.arange(L)
    angt = 2 * np.pi * np.outer(f, t) / L
    basis = np.concatenate([np.cos(angt), -np.sin(angt)], axis=0)          # [64, 2048]
    _CONSTS = (fx, fy, basis.astype(ml_dtypes.bfloat16))
    return _CONSTS


def _host_coeffs(queries, keys):
    """Mean-subtracted compressed coefficients cd [B, L, H, 64], fp32.

    logits[b,h,s,t] = sum_k cd[b,s,h,k] * basis[k,t].
    """
    fx, fy, _ = _constants()
    q4 = queries.reshape(B * L * H, D)
    k4 = keys.reshape(B * L * H, D)
    X = q4 @ fx          # [BLH, 128]
    Y = k4 @ fy
    P = X * Y
    # fold: cos rows = P[0:32] + P[32:64] (re*re + im*im),
    #       -sin rows = P[64:96] - P[96:128] (im*re - re*im)
    Ccs = np.empty((B * L * H, NCC), np.float32)
    np.add(P[:, 0:NF], P[:, NF:2 * NF], out=Ccs[:, 0:NF])
    np.subtract(P[:, 2 * NF:3 * NF], P[:, 3 * NF:4 * NF], out=Ccs[:, NF:NCC])
    Ccs = Ccs.reshape(B, L, H, NCC)
    Ccs -= Ccs.mean(axis=2, keepdims=True)
    return Ccs


def _build_b():
    """Softmax + delay aggregation from host-computed coefficients."""
    exp_op = _register_exp_op()
    nc = bacc.Bacc("TRN2", target_bir_lowering=False, debug=False, num_devices=NCORES)
    cd_d = nc.dram_tensor("cd2", [B, 2 * NCC, L], BF16, kind="ExternalInput")
    # v pre-transposed on host to [B, 128, SC*D]: partition-contiguous rows
    # give 2 KB DMA descriptors instead of 128 B ones
    v_d = nc.dram_tensor("vp", [B, 128, SC * D], BF16, kind="ExternalInput")
    basis_d = nc.dram_tensor("basis2", [2 * NCC, L], BF16, kind="ExternalInput")
    out_d = nc.dram_tensor("out", [B, D, L], BF16, kind="ExternalOutput")

    with tile.TileContext(nc) as tc:
        with (
            tc.tile_pool(name="consts", bufs=1) as consts,
            tc.tile_pool(name="vv", bufs=2) as v_pool,
            tc.tile_pool(name="cd", bufs=4) as cd_pool,
            tc.tile_pool(name="wts", bufs=10) as w_pool,
            tc.tile_pool(name="small", bufs=12) as s_pool,
            tc.tile_pool(name="outp", bufs=2) as out_pool,
            tc.tile_pool(name="ps_log", bufs=3, space="PSUM") as ps_log,
            tc.tile_pool(name="ps_vt", bufs=1, space="PSUM") as ps_vt,
        ):
            # DMA issue order = need order.  Sync ring: cd b0 (first logits
            # stationary), basis halves (first logits moving), cd b1.
            # GpSimd ring in parallel: v0, v1.  Whole-tensor transfers with
            # 4 KB per-partition rows keep descriptor count low.
            cdf0 = cd_pool.tile([2 * NCC, L], BF16, tag="cdf0", name="cdf0")
            basis_sb = consts.tile([2 * NCC, L], BF16)
            # gate chunk 0 on as few bytes as possible: cd-b0 cols 0:1024
            # (first 8 chunks' stationaries) + basis half 0 unblock lg0(0);
            # basis half 1 unblocks lg1(0)
            nc.sync.dma_start(out=cdf0[:, 0:1024], in_=cd_d[0][:, 0:1024])
            nc.sync.dma_start(out=basis_sb[:, 0:1024], in_=basis_d[:, 0:1024])
            nc.sync.dma_start(out=basis_sb[:, 1024:2048], in_=basis_d[:, 1024:2048])
            nc.sync.dma_start(out=cdf0[:, 1024:2048], in_=cd_d[0][:, 1024:2048])
            # v and cd-b1 go on the same ring AFTER the head-gating tensors
    # so their descriptors don't compete for the 16 hw queues during
            # the first-chunk gate; v0 is first needed at iteration ~1.
            v_sbs = []
            v_sb0 = v_pool.tile([128, SC * D], BF16, tag="v0", name="v_sb0")
            nc.sync.dma_start(out=v_sb0[:], in_=v_d[0])
            v_sbs.append(v_sb0)
            cdf1 = cd_pool.tile([2 * NCC, L], BF16, tag="cdf1", name="cdf1")
            nc.sync.dma_start(out=cdf1[:], in_=cd_d[1][:])
            cd_fulls = [cdf0, cdf1]
            v_sb1 = v_pool.tile([128, SC * D], BF16, tag="v1", name="v_sb1")
            nc.gpsimd.dma_start(out=v_sb1[:], in_=v_d[1])
            v_sbs.append(v_sb1)

            for b in range(B):
                v_sb = v_sbs[b]
                vt_ps = ps_vt.tile([128, 1024], F32, tag="vt")
                wts_hist = {}
                vts_hist = {}
                sig_hist = {}

                def emit_acc(sc):
                    pwt = wts_hist.pop(sc)
                    pvts = vts_hist.pop(sc)
                    for q in range(2):
                        # column-packed pair: PE col groups 0:64 / 64:128
                        # co-run with independent moving streams
                        nc.tensor.matmul(
                            vt_ps[0:D, q * 512:(q + 1) * 512],
                            pvts[:],
                            pwt[0][:, q * 512:(q + 1) * 512],
                            start=(sc == 0), stop=(sc == SC - 1),
                        )
                        nc.tensor.matmul(
                            vt_ps[D:2 * D, q * 512:(q + 1) * 512],
                            pvts[:],
                            pwt[1][:, q * 512:(q + 1) * 512],
                            start=(sc == 0), stop=(sc == SC - 1),
                        )

                def emit_small(sc, fast=False):
                    # fast=True only at the pipeline drain (all exps done):
                    # the whole chain runs on the then-idle DVE so the final
                    # aggregation matmuls don't stall on gpsimd's ~1.5 us
                    # latency.  Never route through DVE mid-stream -- any
                    # op inserted ahead of an exp in the in-order queue
                    # cascades into PE stalls.
                    sig = sig_hist.pop(sc)
                    sigsum = s_pool.tile([128, 1], F32, tag="sigsum")
                    eng = nc.vector if fast else nc.gpsimd
                    eng.tensor_add(sigsum[:], sig[:, 0:1], sig[:, 1:2])
                    rcp = s_pool.tile([128, 1], F32, tag="rcp")
                    nc.vector.reciprocal_approx_fast(rcp[:], sigsum[:])
                    vts = s_pool.tile([128, D], BF16, tag="vts")
                    eng.tensor_scalar_mul(
                        vts[:], v_sb[:, sc * D:(sc + 1) * D], rcp[:]
                    )
                    vts_hist[sc] = vts

                for sc in range(SC):
                    cdf = cd_fulls[b]
                    off = sc * 128
                    cdt = cdf[0:NCC, off:off + 128]
                    cdb = cdf[NCC:2 * NCC, off:off + 128]
                    lg0 = ps_log.tile([128, 1024], F32, tag="log")
                    lg1 = ps_log.tile([128, 1024], F32, tag="log")
                    for q in range(2):
                        # alternating PE row-tiles (0,0)/(64,0) keep the
                        # weight-load pipeline overlapped
                        nc.tensor.matmul(
                            lg0[:, q * 512:(q + 1) * 512], cdt,
                            basis_sb[0:NCC, q * 512:(q + 1) * 512],
                            start=True, stop=True,
                        )
                        nc.tensor.matmul(
                            lg1[:, q * 512:(q + 1) * 512], cdb,
                            basis_sb[NCC:2 * NCC, 1024 + q * 512: 1024 + (q + 1) * 512],
                            start=True, stop=True,
                        )
                    if sc >= 3:
                        # lag 3: vts(sc-3) finished a full iteration ago, so
                        # aggregation never waits on the gpsimd scale chain
                        emit_acc(sc - 3)
                    sig = s_pool.tile([128, 2], F32, tag="sig")
                    wt0 = w_pool.tile([128, 1024], BF16, tag="wt")
                    nc.scalar.activation(
                        wt0[:], lg0[:], mybir.ActivationFunctionType.Exp,
                        accum_out=sig[:, 0:1],
                    )
                    wt1 = w_pool.tile([128, 1024], BF16, tag="wt")
                    nc.vector._custom_dve(
                        exp_op, out=wt1[:], in0=lg1[:],
                        s0=EXP_C[0], s1=EXP_C[1], imm2=EXP_C[2],
                        accum_out=sig[:, 1:2],
                    )
                    wts_hist[sc] = (wt0, wt1)
                    sig_hist[sc] = sig
                    if sc >= 1:
                        emit_small(sc - 1)

                emit_small(SC - 1, fast=True)
                emit_acc(SC - 3)
                emit_acc(SC - 2)
                emit_acc(SC - 1)

                out_sb = out_pool.tile([128, 1024], BF16, tag="out")
                # partition-split copies: each out DMA gates on only one copy
                nc.scalar.copy(out_sb[0:D, :], vt_ps[0:D, :])
                nc.sync.dma_start(out=out_d[b][:, 0:1024], in_=out_sb[0:D, :])
                nc.vector.tensor_copy(out_sb[D:2 * D, :], vt_ps[D:2 * D, :])
                nc.sync.dma_start(out=out_d[b][:, 1024:2048], in_=out_sb[D:2 * D, :])
    nc.compile()
    return nc


def _get_compiled():
    global _COMPILED_B
    if _COMPILED_B is None:
        _COMPILED_B = _build_b()
    return _COMPILED_B


def kernel(queries, keys, values):
    global LAST_RESULT
    queries = np.asarray(queries, dtype=np.float32)
    keys = np.asarray(keys, dtype=np.float32)
    values = np.asarray(values, dtype=np.float32)

    _, _, basis16 = _constants()
    bf = ml_dtypes.bfloat16

    cd = _host_coeffs(queries, keys)                  # [B, L, H, 64] fp32
    cdT = np.ascontiguousarray(cd.transpose(0, 2, 3, 1)).astype(bf)  # [B,H,64,L]
    basis2 = np.concatenate([basis16, basis16], axis=0)              # [128, L]

    # v transposed to [B, 128, SC*D]: vp[b, p, c*D + d] = v[b, c*128 + p, d]
    vp_full = values.reshape(B, SC, 128, E).transpose(0, 2, 1, 3)  # [B,128,SC,E]

    in_maps = []
    for i in range(NCORES):
        sl = slice(i * D, (i + 1) * D)
        cd_i = cdT[:, i]                              # [B, 64, L]
        vp_i = np.ascontiguousarray(vp_full[:, :, :, sl]).astype(bf)
        in_maps.append({
            "cd2": np.concatenate([cd_i, cd_i], axis=1),  # [B, 128, L]
            "vp": vp_i.reshape(B, 128, SC * D),
            "basis2": basis2,
        })

    kw = {"trace_cores": list(range(NCORES))} if TRACE else {}
    ncb = _get_compiled()
    res = run_bass_kernel_spmd(ncb, in_maps, core_ids=list(range(NCORES)),
                               trace=TRACE, **kw)
    LAST_RESULT = res

    vt_full = np.stack([res.results[i]["out"] for i in range(NCORES)], axis=1)
    # reference: out = transpose(Vt[B,H,d,L], (0,2,1,3)).reshape(B, L, H*d)
    return np.ascontiguousarray(
        vt_full.astype(np.float32).transpose(0, 2, 1, 3).reshape(B, L, E)
    )
